# revision 36
# baseline (speedup 1.0000x reference)
"""Depth-gated 3x3 conv (DepthConv) Trainium2 Bass kernel.

Problem: out[b,o,h,w] = sum_{c,kh,kw} x[b,c,h+kh-1,w+kw-1]
                        * exp(-|d[b,h,w] - d[b,h+kh-1,w+kw-1]|)
                        * weight[o,c,kh,kw]  + bias[o]
with B=8, Cin=Cout=64, H=W=128, zero padding.

Sharding: data-parallel over batch, one image per NeuronCore (8 cores).

Per-core algorithm (all on-device except pure layout prep of inputs):
  1. gates g[k,s] = exp(-|d_center - d_tap|) computed on 72 partitions
     (9 taps x 8 row-blocks) via DVE sub, DVE abs, ACT exp.
  2. PE "ones-matmul" broadcasts g[k,s] across the 64-channel partition
     dim into PSUM: pg[(tap,c), s] = E.T @ g  (E is a 0/1 selector).
  3. DVE multiplies shifted windows of padded x by pg from PSUM to build
     the gated im2col in SBUF.  x is staged twice (buffers A and B) with
     the upper 64 partitions pre-shifted by +1 / +130 elements so each
     128-partition DVE op covers a PAIR of taps in one pass.
  4. PE GEMM: out[o,s] += wT_chunk.T @ imcol_chunk accumulated over the
     5 chunks (4 tap-pairs + 1 single tap), float32r at full PE rate.
  5. ACT adds bias while copying PSUM->SBUF; DMA to DRAM.
"""

import numpy as np

B, CIN, COUT, H, W = 8, 64, 64, 128, 128
HP, WP = H + 2, W + 2            # padded
NPAD = HP * WP                   # 16900
NXCOL = 16904                    # x staging buffer columns (padded + slack)
S = H * W                        # 16384 outputs per image
NB = 8                           # h-blocks
BH = H // NB                     # 16 rows per block
BLK = BH * W                     # 2048 outputs per block
TW = 512                         # psum tile width (outputs per tile)
NT = S // TW                     # 32 tiles
QT = BLK // TW                   # 4 tiles per block
DCOL = 4352                      # d72 staging columns
DWIN = (BH - 1) * WP + W         # 2078 valid window elems per block
BANDC = 2368                     # band buffer cols (>= 17*130+130)
BANDV = (BH + 1) * WP + W + 2    # 2340 band cols actually loaded

# tap order: chunks j=0..3 are tap pairs, j=4 is the single tap.
# pairs (0,1),(3,4),(6,7) differ by +1 (buffer A), (2,5) by +130 (buffer B)
ORDER = [0, 1, 3, 4, 6, 7, 2, 5, 8]
CHUNK_BUF = ["A", "A", "A", "B", "A"]  # which staging buffer per chunk


def _chunk_low_tap(j):
    return ORDER[2 * j]


def _split_multi_waits(nc, mybir):
    """Walrus in this toolchain encodes at most ONE sync wait per
    instruction.  Tile emits multi-wait sync_info; split the extras into
    single-wait NOPs queued just before on the same engine (identical
    semantics: the engine queue blocks on each wait in turn)."""
    cnt = 0
    for f in nc.m.functions:
        for bb in f.blocks:
            newl = []
            for ins in bb.instructions:
                si = ins.sync_info
                if si is not None and si.on_wait and len(si.on_wait) > 1:
                    waits = list(si.on_wait)
                    for w in waits[:-1]:
                        cnt += 1
                        newl.append(
                            mybir.InstNoOp(
                                name=f"waitsplit-{cnt}",
                                ins=[],
                                outs=[],
                                engine=ins.engine,
                                sync_info=mybir.SyncInfo(on_wait=[w], on_update=[]),
                            )
                        )
                    ins.sync_info = mybir.SyncInfo(
                        on_wait=[waits[-1]], on_update=list(si.on_update)
                    )
                newl.append(ins)
            bb.instructions = newl
    return cnt


def build_nc():
    import concourse.bass as bass
    import concourse.mybir as mybir
    from concourse import tile

    f32 = mybir.dt.float32
    f32r = mybir.dt.float32r
    bf16 = mybir.dt.bfloat16
    Alu = mybir.AluOpType
    Act = mybir.ActivationFunctionType

    nc = bass.Bass()
    xa_d = nc.declare_dram_parameter("xa", [128, NXCOL], f32, isOutput=False)
    xb_d = nc.declare_dram_parameter("xb", [128, NXCOL], f32, isOutput=False)
    d72_d = nc.declare_dram_parameter("d72", [72, DCOL], f32, isOutput=False)
    wt_d = nc.declare_dram_parameter("wt", [640, 64], bf16, isOutput=False)
    em_d = nc.declare_dram_parameter("em", [9, 640], bf16, isOutput=False)
    bias_d = nc.declare_dram_parameter("bias", [64], f32, isOutput=False)
    out_d = nc.declare_dram_parameter("out", [64, S], f32, isOutput=True)

    with tile.TileContext(nc) as tc:
        with (
            tc.tile_pool(name="consts", bufs=1) as consts,
            tc.tile_pool(name="bands", bufs=2) as bands,
            tc.tile_pool(name="imp", bufs=2) as imp,
            tc.tile_pool(name="outp", bufs=3) as outp,
            tc.tile_pool(name="pgp", bufs=6, space=bass.MemorySpace.PSUM) as pgp,
            tc.tile_pool(name="pop", bufs=2, space=bass.MemorySpace.PSUM) as pop,
        ):
            # ---- constants ----
            wt_sb = consts.tile([128, 320], bf16, tag="wt")
            nc.sync.dma_start(
                out=wt_sb[:].rearrange("p (j o) -> p j o", o=64),
                in_=wt_d.rearrange("(j p) o -> p j o", p=128),
            )
            em_sb = consts.tile([9, 640], bf16, tag="em")
            nc.sync.dma_start(out=em_sb[:], in_=em_d[:])
            bias_sb = consts.tile([64, 1], f32, tag="bias")
            nc.sync.dma_start(out=bias_sb[:], in_=bias_d.rearrange("(p o) -> p o", o=1))
            d72_sb = consts.tile([72, DCOL], f32, tag="d72")
            nc.sync.dma_start(out=d72_sb[:], in_=d72_d[:])

            # ---- gates: g = exp(-|d_tap - d_center|) on 72 partitions ----
            gdel = consts.tile([72, BLK], f32, tag="gdel")
            gfin = consts.tile([72, BLK], f32, tag="gfin")
            gexp = consts.tile([72, BLK], bf16, tag="gexp")
            win_s = d72_sb[:, 0:BH * WP].rearrange("q (r w) -> q r w", w=WP)[:, :, :W]
            win_c = d72_sb[:, 2176:2176 + BH * WP].rearrange(
                "q (r w) -> q r w", w=WP
            )[:, :, :W]
            gdel_v = gdel[:].rearrange("q (r w) -> q r w", w=W)
            nc.vector.tensor_sub(gdel_v, win_s, win_c)
            nc.scalar.activation(gfin[:], gdel[:], Act.Abs)
            nc.scalar.activation(gexp[:], gfin[:], Act.Exp, scale=-1.0)
            # relayout [72,2048] -> 8x [9,2048] tiles at partition base 0;
            # each tile has exactly one DMA writer (waits only on exp)
            g9t = []
            for hb in range(NB):
                g9 = consts.tile([9, BLK], bf16, tag=f"g9_{hb}")
                nc.sync.dma_start(out=g9[:], in_=gexp[9 * hb:9 * hb + 9, :])
                g9t.append(g9)

            # ---- main loop over h-blocks ----
            for hb in range(NB):
                banda = bands.tile([128, BANDC], f32, tag="bandA")
                nc.sync.dma_start(
                    out=banda[:, :BANDV], in_=xa_d[:, 2080 * hb:2080 * hb + BANDV]
                )
                bandb = bands.tile([128, BANDC], f32, tag="bandB")
                nc.sync.dma_start(
                    out=bandb[:, :BANDV], in_=xb_d[:, 2080 * hb:2080 * hb + BANDV]
                )
                g9 = g9t[hb][:]

                for qt in range(QT):
                    t = QT * hb + qt
                    # 1) broadcast gates into PSUM via ones-matmul
                    pgs = []
                    for j in range(5):
                        pg = pgp.tile([128, TW], f32, tag="pg")
                        if j < 4:
                            lhs = em_sb[:, 128 * j:128 * j + 128]
                            dst = pg[:, :]
                        else:
                            lhs = em_sb[:, 512:576]
                            dst = pg[0:64, :]
                        nc.tensor.matmul(
                            dst,
                            lhs,
                            g9[:, TW * qt:TW * qt + TW],
                            start=True,
                            stop=True,
                        )
                        pgs.append(pg)
                    # 2) gated im2col chunks (DVE), one pass per chunk
                    ims = []
                    for j in range(5):
                        kh, kw = divmod(_chunk_low_tap(j), 3)
                        off = (4 * qt + kh) * WP + kw
                        band = banda if CHUNK_BUF[j] == "A" else bandb
                        npart = 128 if j < 4 else 64
                        im = imp.tile([npart, TW], bf16, tag=f"im{j}")
                        bw = band[0:npart, off:off + 520].rearrange(
                            "p (r w) -> p r w", w=WP
                        )[:, :4, :W]
                        pgv = pgs[j][0:npart, :].rearrange("p (r w) -> p r w", w=W)
                        imv = im[:].rearrange("p (r w) -> p r w", w=W)
                        nc.vector.tensor_tensor(imv, bw, pgv, Alu.mult)
                        ims.append(im)
                    # 3) main GEMM, accumulate 5 chunks in PSUM
                    po = pop.tile([64, TW], f32, tag="po")
                    for j in range(5):
                        npart = 128 if j < 4 else 64
                        nc.tensor.matmul(
                            po[:],
                            wt_sb[0:npart, 64 * j:64 * j + 64],
                            ims[j][:],
                            start=(j == 0),
                            stop=(j == 4),
                        )
                    # 4) bias add + PSUM->SBUF, then store
                    ot = outp.tile([64, TW], f32, tag="ot")
                    nc.scalar.activation(
                        ot[:], po[:], Act.Identity, bias=bias_sb[:], scale=1.0
                    )
                    nc.sync.dma_start(out=out_d[:, TW * t:TW * t + TW], in_=ot[:])
    _split_multi_waits(nc, mybir)
    return nc


# ---------------- host-side input layout prep ----------------

def _pad_flat(img):
    """[C,H,W] -> [C, NPAD] zero-padded flattened."""
    c = img.shape[0]
    p = np.zeros((c, HP, WP), np.float32)
    p[:, 1:1 + H, 1:1 + W] = img
    return p.reshape(c, NPAD)


def prep_x(x_b):
    """x_b [64,H,W] -> xa, xb [128, NXCOL]: lower=padded x, upper shifted."""
    xp = _pad_flat(np.asarray(x_b, np.float32))
    base = np.zeros((CIN, NXCOL), np.float32)
    base[:, :NPAD] = xp
    upa = np.zeros_like(base)
    upa[:, :NXCOL - 1] = base[:, 1:]
    upb = np.zeros_like(base)
    upb[:, :NXCOL - WP] = base[:, WP:]
    return (np.concatenate([base, upa], 0), np.concatenate([base, upb], 0))


def prep_d(depth_b):
    """depth_b [H,W] -> d72 [72, DCOL] shifted + center depth windows."""
    dp = _pad_flat(np.asarray(depth_b, np.float32)[None])[0]
    d72 = np.zeros((72, DCOL), np.float32)
    for hb in range(NB):
        for k in range(9):
            kh, kw = divmod(k, 3)
            off = 2080 * hb + WP * kh + kw
            d72[9 * hb + k, 0:DWIN] = dp[off:off + DWIN]
            offc = 2080 * hb + WP + 1
            d72[9 * hb + k, 2176:2176 + DWIN] = dp[offc:offc + DWIN]
    return d72


def prep_w(weight):
    """weight [64,64,3,3] -> wt [640,64] chunk-packed, em [9,640] selector."""
    import ml_dtypes

    w2 = np.asarray(weight, np.float32).reshape(COUT, CIN, 9)
    wt = np.zeros((640, 64), ml_dtypes.bfloat16)
    em = np.zeros((9, 640), ml_dtypes.bfloat16)
    for j in range(5):
        for half in range(2 if j < 4 else 1):
            k = ORDER[2 * j + half]
            lo = 128 * j + 64 * half
            wt[lo:lo + 64, :] = w2[:, :, k].T
            em[k, lo:lo + 64] = 1.0
    return wt, em


def make_in_maps(x, depth, weight, bias):
    wt, em = prep_w(weight)
    bias = np.ascontiguousarray(np.asarray(bias, np.float32))
    in_maps = []
    for b in range(B):
        xa, xb = prep_x(x[b])
        d72 = prep_d(np.asarray(depth)[b, 0])
        in_maps.append(
            {"xa": xa, "xb": xb, "d72": d72, "wt": wt, "em": em, "bias": bias}
        )
    return in_maps


_NC = None


def run(x, depth, weight, bias, trace=False):
    global _NC
    from concourse.bass_utils import run_bass_kernel_spmd

    if _NC is None:
        _NC = build_nc()
    in_maps = make_in_maps(x, depth, weight, bias)
    res = run_bass_kernel_spmd(_NC, in_maps, list(range(B)), trace=trace)
    out = np.stack(
        [np.asarray(res.results[b]["out"]).reshape(COUT, H, W) for b in range(B)]
    )
    return out.astype(np.float32), res


def kernel(x, depth, weight, bias):
    out, _ = run(x, depth, weight, bias, trace=False)
    return out


# revision 37
# speedup vs baseline: 1.0903x; 1.0903x over previous
"""Depth-gated 3x3 conv (DepthConv) Trainium2 Bass kernel.

Problem: out[b,o,h,w] = sum_{c,kh,kw} x[b,c,h+kh-1,w+kw-1]
                        * exp(-|d[b,h,w] - d[b,h+kh-1,w+kw-1]|)
                        * weight[o,c,kh,kw]  + bias[o]
with B=8, Cin=Cout=64, H=W=128, zero padding.

Sharding: data-parallel over batch, one image per NeuronCore (8 cores).

Per-core algorithm (all on-device except pure layout prep of inputs):
  1. gates g[k,s] = exp(-|d_center - d_tap|) computed on 72 partitions
     (9 taps x 8 row-blocks) via DVE sub, DVE abs, ACT exp.
  2. PE "ones-matmul" broadcasts g[k,s] across the 64-channel partition
     dim into PSUM: pg[(tap,c), s] = E.T @ g  (E is a 0/1 selector).
  3. DVE multiplies shifted windows of padded x by pg from PSUM to build
     the gated im2col in SBUF.  x is staged twice (buffers A and B) with
     the upper 64 partitions pre-shifted by +1 / +130 elements so each
     128-partition DVE op covers a PAIR of taps in one pass.
  4. PE GEMM: out[o,s] += wT_chunk.T @ imcol_chunk accumulated over the
     5 chunks (4 tap-pairs + 1 single tap), float32r at full PE rate.
  5. ACT adds bias while copying PSUM->SBUF; DMA to DRAM.
"""

import numpy as np

B, CIN, COUT, H, W = 8, 64, 64, 128, 128
HP, WP = H + 2, W + 2            # padded
NPAD = HP * WP                   # 16900
NXCOL = 16904                    # x staging buffer columns (padded + slack)
S = H * W                        # 16384 outputs per image
NB = 8                           # h-blocks
BH = H // NB                     # 16 rows per block
BLK = BH * W                     # 2048 outputs per block
TW = 512                         # psum tile width (outputs per tile)
NT = S // TW                     # 32 tiles
QT = BLK // TW                   # 4 tiles per block
DCOL = 4352                      # d72 staging columns
DWIN = (BH - 1) * WP + W         # 2078 valid window elems per block
BANDC = 2368                     # band buffer cols (>= 17*130+130)
BANDV = (BH + 1) * WP + W + 2    # 2340 band cols actually loaded

# tap order: chunks j=0..3 are tap pairs, j=4 is the single tap.
# pairs (0,1),(3,4),(6,7) differ by +1 (buffer A), (2,5) by +130 (buffer B)
ORDER = [0, 1, 3, 4, 6, 7, 2, 5, 8]
CHUNK_BUF = ["A", "A", "A", "B", "A"]  # which staging buffer per chunk


def _chunk_low_tap(j):
    return ORDER[2 * j]


def _split_multi_waits(nc, mybir):
    """Walrus in this toolchain encodes at most ONE sync wait per
    instruction.  Tile emits multi-wait sync_info; split the extras into
    single-wait NOPs queued just before on the same engine (identical
    semantics: the engine queue blocks on each wait in turn)."""
    cnt = 0
    for f in nc.m.functions:
        for bb in f.blocks:
            newl = []
            for ins in bb.instructions:
                si = ins.sync_info
                if si is not None and si.on_wait and len(si.on_wait) > 1:
                    waits = list(si.on_wait)
                    for w in waits[:-1]:
                        cnt += 1
                        newl.append(
                            mybir.InstNoOp(
                                name=f"waitsplit-{cnt}",
                                ins=[],
                                outs=[],
                                engine=ins.engine,
                                sync_info=mybir.SyncInfo(on_wait=[w], on_update=[]),
                            )
                        )
                    ins.sync_info = mybir.SyncInfo(
                        on_wait=[waits[-1]], on_update=list(si.on_update)
                    )
                newl.append(ins)
            bb.instructions = newl
    return cnt


def build_nc():
    import concourse.bass as bass
    import concourse.mybir as mybir
    from concourse import tile

    f32 = mybir.dt.float32
    f32r = mybir.dt.float32r
    bf16 = mybir.dt.bfloat16
    Alu = mybir.AluOpType
    Act = mybir.ActivationFunctionType

    nc = bass.Bass()
    xa_d = nc.declare_dram_parameter("xa", [128, NXCOL], bf16, isOutput=False)
    xb_d = nc.declare_dram_parameter("xb", [128, NXCOL], bf16, isOutput=False)
    d72_d = nc.declare_dram_parameter("d72", [72, DCOL], f32, isOutput=False)
    wt_d = nc.declare_dram_parameter("wt", [640, 64], bf16, isOutput=False)
    em_d = nc.declare_dram_parameter("em", [9, 640], bf16, isOutput=False)
    bias_d = nc.declare_dram_parameter("bias", [64], f32, isOutput=False)
    out_d = nc.declare_dram_parameter("out", [64, S], f32, isOutput=True)

    with tile.TileContext(nc) as tc:
        with (
            tc.tile_pool(name="consts", bufs=1) as consts,
            tc.tile_pool(name="bands", bufs=2) as bands,
            tc.tile_pool(name="imp", bufs=2) as imp,
            tc.tile_pool(name="gsp", bufs=2) as gsp,
            tc.tile_pool(name="outp", bufs=3) as outp,
            tc.tile_pool(name="pgp", bufs=6, space=bass.MemorySpace.PSUM) as pgp,
            tc.tile_pool(name="pop", bufs=2, space=bass.MemorySpace.PSUM) as pop,
        ):
            # ---- constants ----
            wt_sb = consts.tile([128, 320], bf16, tag="wt")
            nc.sync.dma_start(
                out=wt_sb[:].rearrange("p (j o) -> p j o", o=64),
                in_=wt_d.rearrange("(j p) o -> p j o", p=128),
            )
            em_sb = consts.tile([9, 640], bf16, tag="em")
            nc.sync.dma_start(out=em_sb[:], in_=em_d[:])
            bias_sb = consts.tile([64, 1], f32, tag="bias")
            nc.sync.dma_start(out=bias_sb[:], in_=bias_d.rearrange("(p o) -> p o", o=1))
            d72_sb = consts.tile([72, DCOL], f32, tag="d72")
            nc.sync.dma_start(out=d72_sb[:], in_=d72_d[:])

            # ---- gates: g = exp(-|d_tap - d_center|) on 72 partitions ----
            gdel = consts.tile([72, BLK], f32, tag="gdel")
            gfin = consts.tile([72, BLK], f32, tag="gfin")
            gexp = consts.tile([72, BLK], bf16, tag="gexp")
            win_s = d72_sb[:, 0:BH * WP].rearrange("q (r w) -> q r w", w=WP)[:, :, :W]
            win_c = d72_sb[:, 2176:2176 + BH * WP].rearrange(
                "q (r w) -> q r w", w=WP
            )[:, :, :W]
            gdel_v = gdel[:].rearrange("q (r w) -> q r w", w=W)
            nc.vector.tensor_sub(gdel_v, win_s, win_c)
            nc.scalar.activation(gfin[:], gdel[:], Act.Abs)
            nc.scalar.activation(gexp[:], gfin[:], Act.Exp, scale=-1.0)
            # relayout [72,2048] -> 8x [9,2048] tiles at partition base 0;
            # each tile has exactly one DMA writer (waits only on exp)
            g9t = []
            for hb in range(NB):
                g9 = consts.tile([9, BLK], bf16, tag=f"g9_{hb}")
                nc.sync.dma_start(out=g9[:], in_=gexp[9 * hb:9 * hb + 9, :])
                g9t.append(g9)

            # ---- main loop over h-blocks ----
            for hb in range(NB):
                banda = bands.tile([128, BANDC], bf16, tag="bandA")
                nc.sync.dma_start(
                    out=banda[:, :BANDV], in_=xa_d[:, 2080 * hb:2080 * hb + BANDV]
                )
                bandb = bands.tile([128, BANDC], bf16, tag="bandB")
                nc.sync.dma_start(
                    out=bandb[:, :BANDV], in_=xb_d[:, 2080 * hb:2080 * hb + BANDV]
                )
                g9 = g9t[hb][:]

                for qt in range(QT):
                    t = QT * hb + qt
                    # 1) broadcast gates into PSUM via ones-matmul
                    pgs = []
                    for j in range(5):
                        pg = pgp.tile([128, TW], f32, tag="pg")
                        if j < 4:
                            lhs = em_sb[:, 128 * j:128 * j + 128]
                            dst = pg[:, :]
                        else:
                            lhs = em_sb[:, 512:576]
                            dst = pg[0:64, :]
                        nc.tensor.matmul(
                            dst,
                            lhs,
                            g9[:, TW * qt:TW * qt + TW],
                            start=True,
                            stop=True,
                        )
                        pgs.append(pg)
                    # 2) gates PSUM->SBUF bf16 (split ACT/DVE), then gated
                    #    im2col on DVE in bf16 2x mode
                    gss = []
                    for j in range(5):
                        npart = 128 if j < 4 else 64
                        gs = gsp.tile([npart, TW], bf16, tag=f"gs{j}")
                        if j in (2, 3):
                            nc.vector.tensor_copy(gs[:], pgs[j][0:npart, :])
                        else:
                            nc.scalar.copy(gs[:], pgs[j][0:npart, :])
                        gss.append(gs)
                    ims = []
                    for j in range(5):
                        kh, kw = divmod(_chunk_low_tap(j), 3)
                        off = (4 * qt + kh) * WP + kw
                        band = banda if CHUNK_BUF[j] == "A" else bandb
                        npart = 128 if j < 4 else 64
                        im = imp.tile([npart, TW], bf16, tag=f"im{j}")
                        bw = band[0:npart, off:off + 520].rearrange(
                            "p (r w) -> p r w", w=WP
                        )[:, :4, :W]
                        gsv = gss[j][:].rearrange("p (r w) -> p r w", w=W)
                        imv = im[:].rearrange("p (r w) -> p r w", w=W)
                        nc.vector.tensor_tensor(imv, bw, gsv, Alu.mult)
                        ims.append(im)
                    # 3) main GEMM, accumulate 5 chunks in PSUM
                    po = pop.tile([64, TW], f32, tag="po")
                    for j in range(5):
                        npart = 128 if j < 4 else 64
                        nc.tensor.matmul(
                            po[:],
                            wt_sb[0:npart, 64 * j:64 * j + 64],
                            ims[j][:],
                            start=(j == 0),
                            stop=(j == 4),
                        )
                    # 4) bias add + PSUM->SBUF, then store
                    ot = outp.tile([64, TW], f32, tag="ot")
                    nc.scalar.activation(
                        ot[:], po[:], Act.Identity, bias=bias_sb[:], scale=1.0
                    )
                    nc.sync.dma_start(out=out_d[:, TW * t:TW * t + TW], in_=ot[:])
    _split_multi_waits(nc, mybir)
    return nc


# ---------------- host-side input layout prep ----------------

def _pad_flat(img):
    """[C,H,W] -> [C, NPAD] zero-padded flattened."""
    c = img.shape[0]
    p = np.zeros((c, HP, WP), np.float32)
    p[:, 1:1 + H, 1:1 + W] = img
    return p.reshape(c, NPAD)


def prep_x(x_b):
    """x_b [64,H,W] -> xa, xb [128, NXCOL] bf16: lower=padded x, upper
    shifted by +1 / +WP elements."""
    import ml_dtypes

    xp = _pad_flat(np.asarray(x_b, np.float32))
    base = np.zeros((CIN, NXCOL), np.float32)
    base[:, :NPAD] = xp
    upa = np.zeros_like(base)
    upa[:, :NXCOL - 1] = base[:, 1:]
    upb = np.zeros_like(base)
    upb[:, :NXCOL - WP] = base[:, WP:]
    bf = ml_dtypes.bfloat16
    return (
        np.concatenate([base, upa], 0).astype(bf),
        np.concatenate([base, upb], 0).astype(bf),
    )


def prep_d(depth_b):
    """depth_b [H,W] -> d72 [72, DCOL] shifted + center depth windows."""
    dp = _pad_flat(np.asarray(depth_b, np.float32)[None])[0]
    d72 = np.zeros((72, DCOL), np.float32)
    for hb in range(NB):
        for k in range(9):
            kh, kw = divmod(k, 3)
            off = 2080 * hb + WP * kh + kw
            d72[9 * hb + k, 0:DWIN] = dp[off:off + DWIN]
            offc = 2080 * hb + WP + 1
            d72[9 * hb + k, 2176:2176 + DWIN] = dp[offc:offc + DWIN]
    return d72


def prep_w(weight):
    """weight [64,64,3,3] -> wt [640,64] chunk-packed, em [9,640] selector."""
    import ml_dtypes

    w2 = np.asarray(weight, np.float32).reshape(COUT, CIN, 9)
    wt = np.zeros((640, 64), ml_dtypes.bfloat16)
    em = np.zeros((9, 640), ml_dtypes.bfloat16)
    for j in range(5):
        for half in range(2 if j < 4 else 1):
            k = ORDER[2 * j + half]
            lo = 128 * j + 64 * half
            wt[lo:lo + 64, :] = w2[:, :, k].T
            em[k, lo:lo + 64] = 1.0
    return wt, em


def make_in_maps(x, depth, weight, bias):
    wt, em = prep_w(weight)
    bias = np.ascontiguousarray(np.asarray(bias, np.float32))
    in_maps = []
    for b in range(B):
        xa, xb = prep_x(x[b])
        d72 = prep_d(np.asarray(depth)[b, 0])
        in_maps.append(
            {"xa": xa, "xb": xb, "d72": d72, "wt": wt, "em": em, "bias": bias}
        )
    return in_maps


_NC = None


def run(x, depth, weight, bias, trace=False):
    global _NC
    from concourse.bass_utils import run_bass_kernel_spmd

    if _NC is None:
        _NC = build_nc()
    in_maps = make_in_maps(x, depth, weight, bias)
    res = run_bass_kernel_spmd(_NC, in_maps, list(range(B)), trace=trace)
    out = np.stack(
        [np.asarray(res.results[b]["out"]).reshape(COUT, H, W) for b in range(B)]
    )
    return out.astype(np.float32), res


def kernel(x, depth, weight, bias):
    out, _ = run(x, depth, weight, bias, trace=False)
    return out


# revision 41
# speedup vs baseline: 1.4209x; 1.3032x over previous
"""Depth-gated 3x3 conv (DepthConv) Trainium2 Bass kernel.

Problem: out[b,o,h,w] = sum_{c,kh,kw} x[b,c,h+kh-1,w+kw-1]
                        * exp(-|d[b,h,w] - d[b,h+kh-1,w+kw-1]|)
                        * weight[o,c,kh,kw]  + bias[o]
with B=8, Cin=Cout=64, H=W=128, zero padding.

Sharding: data-parallel over batch, one image per NeuronCore (8 cores).

Per-core algorithm (all on-device except pure layout prep of inputs):
  1. gates g[k,s] = exp(-|d_center - d_tap|) computed on 72 partitions
     (9 taps x 8 row-blocks) via DVE sub, DVE abs, ACT exp.
  2. PE "ones-matmul" broadcasts g[k,s] across the 64-channel partition
     dim into PSUM: pg[(tap,c), s] = E.T @ g  (E is a 0/1 selector).
  3. DVE multiplies shifted windows of padded x by pg from PSUM to build
     the gated im2col in SBUF.  x is staged twice (buffers A and B) with
     the upper 64 partitions pre-shifted by +1 / +130 elements so each
     128-partition DVE op covers a PAIR of taps in one pass.
  4. PE GEMM: out[o,s] += wT_chunk.T @ imcol_chunk accumulated over the
     5 chunks (4 tap-pairs + 1 single tap), float32r at full PE rate.
  5. ACT adds bias while copying PSUM->SBUF; DMA to DRAM.
"""

import numpy as np

B, CIN, COUT, H, W = 8, 64, 64, 128, 128
HP, WP = H + 2, W + 2            # padded
NPAD = HP * WP                   # 16900
NXCOL = 16904                    # x staging buffer columns (padded + slack)
S = H * W                        # 16384 outputs per image
NB = 8                           # h-blocks
BH = H // NB                     # 16 rows per block
BLK = BH * W                     # 2048 outputs per block
TW = 512                         # psum tile width (outputs per tile)
NT = S // TW                     # 32 tiles
QT = BLK // TW                   # 4 tiles per block
DCOL = 4352                      # d72 staging columns
DWIN = (BH - 1) * WP + W         # 2078 valid window elems per block
BANDC = 2368                     # band buffer cols (>= 17*130+130)
BANDV = (BH + 1) * WP + W + 2    # 2340 band cols actually loaded

# tap order: chunks j=0..3 are tap pairs, j=4 is the single tap.
# pairs (0,1),(3,4),(6,7) differ by +1 (buffer A), (2,5) by +130 (buffer B)
ORDER = [0, 1, 3, 4, 6, 7, 2, 5, 8]
CHUNK_BUF = ["A", "A", "A", "B", "A"]  # which staging buffer per chunk


def _chunk_low_tap(j):
    return ORDER[2 * j]


def _split_multi_waits(nc, mybir):
    """Walrus in this toolchain encodes at most ONE sync wait per
    instruction.  Tile emits multi-wait sync_info; split the extras into
    single-wait NOPs queued just before on the same engine (identical
    semantics: the engine queue blocks on each wait in turn)."""
    cnt = 0
    for f in nc.m.functions:
        for bb in f.blocks:
            newl = []
            for ins in bb.instructions:
                si = ins.sync_info
                if si is not None and si.on_wait and len(si.on_wait) > 1:
                    waits = list(si.on_wait)
                    for w in waits[:-1]:
                        cnt += 1
                        newl.append(
                            mybir.InstNoOp(
                                name=f"waitsplit-{cnt}",
                                ins=[],
                                outs=[],
                                engine=ins.engine,
                                sync_info=mybir.SyncInfo(on_wait=[w], on_update=[]),
                            )
                        )
                    ins.sync_info = mybir.SyncInfo(
                        on_wait=[waits[-1]], on_update=list(si.on_update)
                    )
                newl.append(ins)
            bb.instructions = newl
    return cnt


def build_nc():
    import concourse.bass as bass
    import concourse.mybir as mybir
    from concourse import tile

    f32 = mybir.dt.float32
    f32r = mybir.dt.float32r
    bf16 = mybir.dt.bfloat16
    Alu = mybir.AluOpType
    Act = mybir.ActivationFunctionType

    nc = bass.Bass()
    xa_d = nc.declare_dram_parameter("xa", [128, NXCOL], bf16, isOutput=False)
    xb_d = nc.declare_dram_parameter("xb", [128, NXCOL], bf16, isOutput=False)
    d72_d = nc.declare_dram_parameter("d72", [72, DCOL], f32, isOutput=False)
    wt_d = nc.declare_dram_parameter("wt", [640, 64], bf16, isOutput=False)
    em_d = nc.declare_dram_parameter("em", [128, 640], bf16, isOutput=False)
    bias_d = nc.declare_dram_parameter("bias", [128], f32, isOutput=False)
    out_d = nc.declare_dram_parameter("out", [64, S], f32, isOutput=True)

    with tile.TileContext(nc) as tc:
        with (
            tc.tile_pool(name="consts", bufs=1) as consts,
            tc.tile_pool(name="bands", bufs=2) as bands,
            tc.tile_pool(name="imp", bufs=2) as imp,
            tc.tile_pool(name="gsp", bufs=2) as gsp,
            tc.tile_pool(name="outp", bufs=3) as outp,
            tc.tile_pool(name="pgp", bufs=6, space=bass.MemorySpace.PSUM) as pgp,
            tc.tile_pool(name="pop", bufs=2, space=bass.MemorySpace.PSUM) as pop,
        ):
            # ---- constants ----
            wt_sb = consts.tile([128, 320], bf16, tag="wt")
            nc.sync.dma_start(
                out=wt_sb[:].rearrange("p (j o) -> p j o", o=64),
                in_=wt_d.rearrange("(j p) o -> p j o", p=128),
            )
            em_sb = consts.tile([128, 640], bf16, tag="em")
            nc.sync.dma_start(out=em_sb[:], in_=em_d[:])
            bias_sb = consts.tile([128, 1], f32, tag="bias")
            nc.sync.dma_start(out=bias_sb[:], in_=bias_d.rearrange("(p o) -> p o", o=1))
            d72_sb = consts.tile([72, DCOL], f32, tag="d72")
            nc.sync.dma_start(out=d72_sb[:], in_=d72_d[:])

            # ---- gates: g = exp(-|d_tap - d_center|) on 72 partitions ----
            gdel = consts.tile([72, BLK], f32, tag="gdel")
            gfin = consts.tile([72, BLK], f32, tag="gfin")
            gexp = consts.tile([72, BLK], bf16, tag="gexp")
            win_s = d72_sb[:, 0:BH * WP].rearrange("q (r w) -> q r w", w=WP)[:, :, :W]
            win_c = d72_sb[:, 2176:2176 + BH * WP].rearrange(
                "q (r w) -> q r w", w=WP
            )[:, :, :W]
            gdel_v = gdel[:].rearrange("q (r w) -> q r w", w=W)
            nc.vector.tensor_sub(gdel_v, win_s, win_c)
            nc.scalar.activation(gfin[:], gdel[:], Act.Abs)
            nc.scalar.activation(gexp[:], gfin[:], Act.Exp, scale=-1.0)
            # relayout [72,2048] -> 8x [9,2048] tiles at partition base 0;
            # each tile has exactly one DMA writer (waits only on exp)
            g9t = []
            for hb in range(NB):
                g9 = consts.tile([128, BLK], bf16, tag=f"g9_{hb}")
                for r in range(4):
                    nc.sync.dma_start(
                        out=g9[32 * r:32 * r + 9, :],
                        in_=gexp[9 * hb:9 * hb + 9, :],
                    )
                g9t.append(g9)

            # ---- main loop over h-blocks ----
            for hb in range(NB):
                banda = bands.tile([128, BANDC], bf16, tag="bandA")
                nc.sync.dma_start(
                    out=banda[:, :BANDV], in_=xa_d[:, 2080 * hb:2080 * hb + BANDV]
                )
                bandb = bands.tile([128, BANDC], bf16, tag="bandB")
                nc.sync.dma_start(
                    out=bandb[:, :BANDV], in_=xb_d[:, 2080 * hb:2080 * hb + BANDV]
                )
                g9 = g9t[hb][:]

                for q2 in range(QT // 2):
                    # two psum-tiles (1024 outputs) per group
                    t0 = QT * hb + 2 * q2
                    # 1) gates -> PSUM f32 [128,512] x2 waves; each wave is
                    #    4 row-packed ones-matmuls + one single (chunk 4)
                    pgs = [[None] * 5, [None] * 5]
                    for w in range(2):
                        qt = 2 * q2 + w
                        for j in range(4):
                            pg = pgp.tile([128, TW], f32, tag="pg")
                            nc.tensor.matmul(
                                pg[:, :],
                                em_sb[32 * j:32 * j + 9, 128 * j:128 * j + 128],
                                g9[32 * j:32 * j + 9, TW * qt:TW * (qt + 1)],
                                start=True,
                                stop=True,
                                tile_position=(32 * j, 0),
                            )
                            pgs[w][j] = pg
                        pg4 = pgp.tile([128, TW], f32, tag="pg")
                        nc.tensor.matmul(
                            pg4[0:64, :],
                            em_sb[0:9, 512:576],
                            g9[0:9, TW * qt:TW * (qt + 1)],
                            start=True,
                            stop=True,
                            tile_position=(0, 0),
                        )
                        pgs[w][4] = pg4
                    # 2) chunks 0,1,4: ACT copy psum->SBUF bf16 then one wide
                    #    2x TT; chunks 2,3: DVE TT straight from PSUM (1x)
                    gss = {}
                    for j in (0, 1, 4):
                        npart = 128 if j < 4 else 64
                        gs = gsp.tile([npart, 2 * TW], bf16, tag=f"gs{j}")
                        nc.scalar.copy(gs[:, 0:TW], pgs[0][j][0:npart, :])
                        nc.scalar.copy(gs[:, TW:2 * TW], pgs[1][j][0:npart, :])
                        gss[j] = gs
                    # 3) gated im2col into bf16 SBUF
                    ims = []
                    for j in range(5):
                        kh, kw = divmod(_chunk_low_tap(j), 3)
                        band = banda if CHUNK_BUF[j] == "A" else bandb
                        npart = 128 if j < 4 else 64
                        im = imp.tile([npart, 2 * TW], bf16, tag=f"im{j}")
                        if j in (2, 3):
                            for w in range(2):
                                off = ((8 * q2 + 4 * w) + kh) * WP + kw
                                bw = band[0:npart, off:off + 520].rearrange(
                                    "p (r w) -> p r w", w=WP
                                )[:, :4, :W]
                                pgv = pgs[w][j][0:npart, :].rearrange(
                                    "p (r w) -> p r w", w=W
                                )
                                imv = im[:, TW * w:TW * (w + 1)].rearrange(
                                    "p (r w) -> p r w", w=W
                                )
                                nc.vector.tensor_tensor(imv, bw, pgv, Alu.mult)
                        else:
                            off = (8 * q2 + kh) * WP + kw
                            bw = band[0:npart, off:off + 1040].rearrange(
                                "p (r w) -> p r w", w=WP
                            )[:, :8, :W]
                            gsv = gss[j][:].rearrange("p (r w) -> p r w", w=W)
                            imv = im[:].rearrange("p (r w) -> p r w", w=W)
                            nc.vector.tensor_tensor(imv, bw, gsv, Alu.mult)
                        ims.append(im)
                    # 4) main GEMM: col-paired across the two 512-tiles
                    po = pop.tile([128, TW], f32, tag="po")
                    for j in range(5):
                        npart = 128 if j < 4 else 64
                        lhs = wt_sb[0:npart, 64 * j:64 * j + 64]
                        nc.tensor.matmul(
                            po[0:64, :],
                            lhs,
                            ims[j][:, 0:TW],
                            start=(j == 0),
                            stop=(j == 4),
                            tile_position=(0, 0),
                            skip_group_check=True,
                        )
                        nc.tensor.matmul(
                            po[64:128, :],
                            lhs,
                            ims[j][:, TW:2 * TW],
                            start=(j == 0),
                            stop=(j == 4),
                            tile_position=(0, 64),
                            skip_group_check=True,
                        )
                    # 5) bias add + store both tiles
                    ot = outp.tile([128, TW], f32, tag="ot")
                    nc.scalar.activation(
                        ot[:], po[:], Act.Identity, bias=bias_sb[:], scale=1.0
                    )
                    nc.sync.dma_start(
                        out=out_d[:, TW * t0:TW * (t0 + 1)], in_=ot[0:64, :]
                    )
                    nc.sync.dma_start(
                        out=out_d[:, TW * (t0 + 1):TW * (t0 + 2)], in_=ot[64:128, :]
                    )
    _split_multi_waits(nc, mybir)
    return nc


# ---------------- host-side input layout prep ----------------

def _pad_flat(img):
    """[C,H,W] -> [C, NPAD] zero-padded flattened."""
    c = img.shape[0]
    p = np.zeros((c, HP, WP), np.float32)
    p[:, 1:1 + H, 1:1 + W] = img
    return p.reshape(c, NPAD)


def prep_x(x_b):
    """x_b [64,H,W] -> xa, xb [128, NXCOL] bf16: lower=padded x, upper
    shifted by +1 / +WP elements."""
    import ml_dtypes

    xp = _pad_flat(np.asarray(x_b, np.float32))
    base = np.zeros((CIN, NXCOL), np.float32)
    base[:, :NPAD] = xp
    upa = np.zeros_like(base)
    upa[:, :NXCOL - 1] = base[:, 1:]
    upb = np.zeros_like(base)
    upb[:, :NXCOL - WP] = base[:, WP:]
    bf = ml_dtypes.bfloat16
    return (
        np.concatenate([base, upa], 0).astype(bf),
        np.concatenate([base, upb], 0).astype(bf),
    )


def prep_d(depth_b):
    """depth_b [H,W] -> d72 [72, DCOL] shifted + center depth windows."""
    dp = _pad_flat(np.asarray(depth_b, np.float32)[None])[0]
    d72 = np.zeros((72, DCOL), np.float32)
    for hb in range(NB):
        for k in range(9):
            kh, kw = divmod(k, 3)
            off = 2080 * hb + WP * kh + kw
            d72[9 * hb + k, 0:DWIN] = dp[off:off + DWIN]
            offc = 2080 * hb + WP + 1
            d72[9 * hb + k, 2176:2176 + DWIN] = dp[offc:offc + DWIN]
    return d72


def prep_w(weight):
    """weight [64,64,3,3] -> wt [640,64] chunk-packed, em4 [128,640]
    selector replicated at row-group bases 0/32/64/96."""
    import ml_dtypes

    w2 = np.asarray(weight, np.float32).reshape(COUT, CIN, 9)
    wt = np.zeros((640, 64), ml_dtypes.bfloat16)
    em = np.zeros((9, 640), np.float32)
    for j in range(5):
        for half in range(2 if j < 4 else 1):
            k = ORDER[2 * j + half]
            lo = 128 * j + 64 * half
            wt[lo:lo + 64, :] = w2[:, :, k].T
            em[k, lo:lo + 64] = 1.0
    em4 = np.zeros((128, 640), ml_dtypes.bfloat16)
    for r in range(4):
        em4[32 * r:32 * r + 9, :] = em
    return wt, em4


def make_in_maps(x, depth, weight, bias):
    wt, em4 = prep_w(weight)
    bias2 = np.ascontiguousarray(np.tile(np.asarray(bias, np.float32), 2))
    in_maps = []
    for b in range(B):
        xa, xb = prep_x(x[b])
        d72 = prep_d(np.asarray(depth)[b, 0])
        in_maps.append(
            {"xa": xa, "xb": xb, "d72": d72, "wt": wt, "em": em4, "bias": bias2}
        )
    return in_maps


_NC = None


def run(x, depth, weight, bias, trace=False):
    global _NC
    from concourse.bass_utils import run_bass_kernel_spmd

    if _NC is None:
        _NC = build_nc()
    in_maps = make_in_maps(x, depth, weight, bias)
    res = run_bass_kernel_spmd(_NC, in_maps, list(range(B)), trace=trace)
    out = np.stack(
        [np.asarray(res.results[b]["out"]).reshape(COUT, H, W) for b in range(B)]
    )
    return out.astype(np.float32), res


def kernel(x, depth, weight, bias):
    out, _ = run(x, depth, weight, bias, trace=False)
    return out
